# revision 18
# baseline (speedup 1.0000x reference)
"""CurvGN-style 2-layer GNN message passing on 8 TRN2 NeuronCores (Bass/Tile).

Contract: kernel(**inputs) takes the FULL unsharded inputs (as produced by
setup_inputs) and returns the FULL [50000, 40] float32 output.

Math per layer (channels C):
    h   = x @ W + b                      [N, C]
    ow  = prelu(w_mul @ w0, a) @ w1      [E, C]  (the per-channel bias of this
                                          MLP cancels in the segment softmax)
    p   = exp(ow)        (segment-max subtraction dropped: |ow| is O(1), exp
                          is well-conditioned, softmax is shift-invariant)
    den = segment_sum(p, src);  g = h / den
    msg = p * g[src];   agg = segment_sum(msg, dst)
Layer1 -> elu -> Layer2 -> log_softmax(rows).

Sharding: nodes are padded to N_PAD = 8*OWN; core k owns node range
[k*OWN, (k+1)*OWN) and ALL edges whose src lies in it, so den/g are
core-local.  Each core writes a partial agg over all N_PAD nodes; one
ReduceScatter per layer hands every core the rows it owns.

Per layer each core runs two passes over its edges (128-edge chunks):
  pass A (chunks grouped by 128-node src tile): edge-MLP -> exp (pad edges
      get bias -80 so exp ~ 0) -> onehot-matmul segment-sum accumulated in
      PSUM per src tile -> den.
  pass B (chunks grouped by 128-node dst block): edge-MLP recomputed in
      dst order -> exp -> indirect-DMA row gather of g by src -> multiply ->
      onehot-matmul scatter accumulated in PSUM per dst block -> static DMA
      into the partial agg buffer.

All per-edge layout data (slot assignment, onehot targets, gather indices,
pad masks) is computed on host from edge_index; the chunk structure is
max-padded across cores so a single SPMD program serves all 8 cores.
"""

import sys

if "/opt/trn_rl_repo" not in sys.path:
    sys.path.insert(0, "/opt/trn_rl_repo")

import numpy as np

import concourse.bass as bass
import concourse.tile as tile
from concourse import bacc, mybir
from concourse.masks import make_identity

F32 = mybir.dt.float32
I32 = mybir.dt.int32
AF = mybir.ActivationFunctionType
ALU = mybir.AluOpType

NCORES = 8
PAD_BIAS = -80.0  # exp(-80) ~ 1.8e-35: pad edges contribute ~0


def _ceil(a, b):
    return -(-a // b)


# ----------------------------------------------------------------------------
# host-side preprocessing
# ----------------------------------------------------------------------------

def build_meta_and_inputs(inputs, ncores):
    x = np.asarray(inputs["x"], np.float32)
    ei = np.asarray(inputs["edge_index"]).astype(np.int64)
    w_mul = np.asarray(inputs["w_mul"], np.float32)
    n_nodes, f_in = x.shape
    n_edges, w_dim = w_mul.shape
    src, dst = ei[0], ei[1]

    own = _ceil(n_nodes, ncores * 128) * 128
    n_pad = own * ncores
    ntile = own // 128
    nblk = n_pad // 128

    core_of_edge = src // own
    tile_of_edge = (src - core_of_edge * own) // 128
    blk_of_edge = dst // 128

    cntA = np.zeros((ncores, ntile), np.int64)
    cntB = np.zeros((ncores, nblk), np.int64)
    for k in range(ncores):
        m = core_of_edge == k
        cntA[k] = np.bincount(tile_of_edge[m], minlength=ntile)
        cntB[k] = np.bincount(blk_of_edge[m], minlength=nblk)

    chA = _ceil(cntA.max(axis=0), 128)
    chB = _ceil(cntB.max(axis=0), 128)
    CH_A = int(chA.sum())
    CH_B = int(chB.sum())
    chunkA_tile = np.repeat(np.arange(ntile), chA)
    chunkB_blk = np.repeat(np.arange(nblk), chB)
    offA = np.concatenate([[0], np.cumsum(chA)])
    offB = np.concatenate([[0], np.cumsum(chB)])

    meta = dict(
        n_nodes=n_nodes, n_pad=n_pad, own=own, ntile=ntile, nblk=nblk,
        f_in=f_in, w_dim=w_dim, n_edges=n_edges,
        CH_A=CH_A, CH_B=CH_B,
        chA=chA, chB=chB, offA=offA, offB=offB,
        chunkA_tile=chunkA_tile, chunkB_blk=chunkB_blk,
        ncores=ncores,
    )

    in_maps = []
    order = np.arange(n_edges)
    for k in range(ncores):
        m = core_of_edge == k
        ek = order[m]
        # pass A
        tA = tile_of_edge[ek]
        sA = np.argsort(tA, kind="stable")
        ekA, tAs = ek[sA], tA[sA]
        startsA = np.searchsorted(tAs, np.arange(ntile))
        rankA = np.arange(len(ekA)) - startsA[tAs]
        slotA = offA[tAs] * 128 + rankA
        nsA = CH_A * 128
        wmulT_A = np.zeros((w_dim, nsA), np.float32)
        srcrel_A = np.zeros(nsA, np.float32)
        maskb_A = np.full(nsA, PAD_BIAS, np.float32)
        wmulT_A[:, slotA] = w_mul[ekA].T
        srcrel_A[slotA] = (src[ekA] - k * own - tAs * 128).astype(np.float32)
        maskb_A[slotA] = 0.0
        # pass B
        bB = blk_of_edge[ek]
        sB = np.argsort(bB, kind="stable")
        ekB, bBs = ek[sB], bB[sB]
        startsB = np.searchsorted(bBs, np.arange(nblk))
        rankB = np.arange(len(ekB)) - startsB[bBs]
        slotB = offB[bBs] * 128 + rankB
        nsB = CH_B * 128
        wmulT_B = np.zeros((w_dim, nsB), np.float32)
        dstrel_B = np.zeros(nsB, np.float32)
        gidx_B = np.zeros(nsB, np.int32)
        maskb_B = np.full(nsB, PAD_BIAS, np.float32)
        wmulT_B[:, slotB] = w_mul[ekB].T
        dstrel_B[slotB] = (dst[ekB] - bBs * 128).astype(np.float32)
        gidx_B[slotB] = (src[ekB] - k * own).astype(np.int32)
        maskb_B[slotB] = 0.0

        lo = k * own
        hi = min((k + 1) * own, n_nodes)
        xT = np.zeros((f_in, own), np.float32)
        xT[:, : hi - lo] = x[lo:hi].T

        # int16 index table for dma_gather: groups of GROUP chunks; index i
        # of a group lands at [i % 16, group_col_base + i // 16], pattern
        # replicated across all 128 partitions (16 rows per Q7 core).
        GROUP = 4
        cols16 = []
        for c0 in range(0, CH_B, GROUP):
            gsz = min(GROUP, CH_B - c0)
            blk = gidx_B[c0 * 128:(c0 + gsz) * 128].reshape(gsz * 8, 16).T
            cols16.append(blk)
        gidx16 = np.tile(np.concatenate(cols16, axis=1), (8, 1)) \
            .astype(np.int16)

        def cmaj(a, ch):
            return np.ascontiguousarray(a.reshape(ch, 128).T)

        in_maps.append({
            "xT": np.ascontiguousarray(xT),
            "wmulT_A": wmulT_A, "wmulT_B": wmulT_B,
            "srcrel_A": cmaj(srcrel_A, CH_A),
            "maskb_A": cmaj(maskb_A, CH_A),
            "dstrel_B": cmaj(dstrel_B, CH_B),
            "gidx_B": cmaj(gidx_B, CH_B),
            "gidx16_B": np.ascontiguousarray(gidx16),
            "maskb_B": cmaj(maskb_B, CH_B),
        })

    hid = np.asarray(inputs["m1_w0"]).shape[1]
    ncls = np.asarray(inputs["m2_w0"]).shape[1]
    meta["hid"] = hid
    meta["ncls"] = ncls
    meta["prelu"] = (
        "abs" if (np.all(np.asarray(inputs["m1_a"]) <= 1.0)
                  and np.all(np.asarray(inputs["m2_a"]) <= 1.0)) else "dve")
    rep = {
        "W1": np.asarray(inputs["W1"], np.float32),
        "b1": np.asarray(inputs["b1"], np.float32).reshape(1, -1),
        "m1_w0": np.asarray(inputs["m1_w0"], np.float32),
        "m1_a": np.ascontiguousarray(
            np.asarray(inputs["m1_a"], np.float32).reshape(-1, 128).T),
        "m1_w1": np.asarray(inputs["m1_w1"], np.float32),
        "W2": np.asarray(inputs["W2"], np.float32),
        "b2": np.asarray(inputs["b2"], np.float32).reshape(1, -1),
        "m2_w0": np.asarray(inputs["m2_w0"], np.float32),
        "m2_a": np.ascontiguousarray(
            np.pad(np.asarray(inputs["m2_a"], np.float32), (0, 128 - ncls))
            .reshape(-1, 128).T),
        "m2_w1": np.asarray(inputs["m2_w1"], np.float32),
    }
    for im in in_maps:
        im.update(rep)
    return meta, in_maps


# ----------------------------------------------------------------------------
# device kernel
# ----------------------------------------------------------------------------

def build_kernel(meta):
    ncores = meta["ncores"]
    f_in, w_dim, hid, ncls = (meta[s] for s in ("f_in", "w_dim", "hid",
                                                "ncls"))
    own = meta["own"]
    CH_A, CH_B = meta["CH_A"], meta["CH_B"]

    nc = bacc.Bacc("TRN2", target_bir_lowering=False, debug=False,
                   num_devices=ncores)

    def din(name, shape, dt=F32):
        return nc.dram_tensor(name, list(shape), dt, kind="ExternalInput").ap()

    xT = din("xT", [f_in, own])
    wmulT_A = din("wmulT_A", [w_dim, CH_A * 128])
    wmulT_B = din("wmulT_B", [w_dim, CH_B * 128])
    srcrel_A = din("srcrel_A", [128, CH_A])
    maskb_A = din("maskb_A", [128, CH_A])
    dstrel_B = din("dstrel_B", [128, CH_B])
    gidx_B = din("gidx_B", [128, CH_B], I32)
    gidx16_B = din("gidx16_B", [128, CH_B * 8], mybir.dt.int16)
    maskb_B = din("maskb_B", [128, CH_B])
    W1 = din("W1", [f_in, hid])
    b1 = din("b1", [1, hid])
    m1_w0 = din("m1_w0", [w_dim, hid])
    m1_a = din("m1_a", [128, max(1, hid // 128)])
    m1_w1 = din("m1_w1", [hid, hid])
    W2 = din("W2", [hid, ncls])
    b2 = din("b2", [1, ncls])
    m2_w0 = din("m2_w0", [w_dim, ncls])
    m2_a = din("m2_a", [128, 1])
    m2_w1 = din("m2_w1", [ncls, ncls])
    out = nc.dram_tensor("out", [own, ncls], F32, kind="ExternalOutput").ap()

    io = locals()
    repeat = meta.get("repeat", 1)
    with tile.TileContext(nc) as tc:
        with tc.tile_pool(name="dram", bufs=1, space="DRAM") as dram, \
             tc.tile_pool(name="const", bufs=1) as cpool:
            for _r in range(repeat):
                _build_body(nc, tc, meta, io, dram, cpool)
    nc.compile()
    return nc


def _build_body(nc, tc, meta, io, dram, cpool):
    ncores = meta["ncores"]
    n_pad, own, ntile, nblk = (meta[s] for s in ("n_pad", "own", "ntile",
                                                 "nblk"))
    f_in, w_dim, hid, ncls = (meta[s] for s in ("f_in", "w_dim", "hid",
                                                "ncls"))
    CH_A, CH_B = meta["CH_A"], meta["CH_B"]
    chA, chB, offA, offB = meta["chA"], meta["chB"], meta["offA"], meta["offB"]
    chunkA_tile, chunkB_blk = meta["chunkA_tile"], meta["chunkB_blk"]
    GROUP = 4
    rg = [list(range(ncores))]

    xT, W1, b1, W2, b2 = (io[s] for s in ("xT", "W1", "b1", "W2", "b2"))
    out = io["out"]

    h1_d = dram.tile([own, hid], F32)
    den1_d = dram.tile([own, hid], F32)
    g1_d = dram.tile([own, hid], F32)
    pagg1_d = dram.tile([n_pad, hid], F32)
    agg1_d = dram.tile([own, hid], F32)
    h2_d = dram.tile([own, 64], F32)
    den2_d = dram.tile([own, ncls], F32)
    g2_d = dram.tile([own, 64], F32)
    pagg2_d = dram.tile([n_pad, ncls], F32)
    agg2_d = dram.tile([own, ncls], F32)

    kt1 = f_in // 128
    W1_s = cpool.tile([128, kt1 * hid], F32)
    for i in range(kt1):
        nc.sync.dma_start(out=W1_s[:, i * hid:(i + 1) * hid],
                          in_=W1[i * 128:(i + 1) * 128, :])
    b1_s = cpool.tile([1, hid], F32)
    nc.sync.dma_start(out=b1_s[:], in_=b1[:])
    ht1 = (hid + 127) // 128
    m1w0_s = cpool.tile([w_dim, hid], F32)
    nc.sync.dma_start(out=m1w0_s[:], in_=io["m1_w0"][:])
    m1a_s = cpool.tile([128, ht1], F32)
    nc.sync.dma_start(out=m1a_s[:], in_=io["m1_a"][:])
    m1w1_s = cpool.tile([128, ht1 * hid], F32)
    for i in range(ht1):
        nc.sync.dma_start(out=m1w1_s[:, i * hid:(i + 1) * hid],
                          in_=io["m1_w1"][i * 128:(i + 1) * 128, :])
    W2_s = cpool.tile([128, (hid // 128) * ncls], F32)
    for i in range(hid // 128):
        nc.sync.dma_start(out=W2_s[:, i * ncls:(i + 1) * ncls],
                          in_=io["W2"][i * 128:(i + 1) * 128, :])
    b2_s = cpool.tile([1, ncls], F32)
    nc.sync.dma_start(out=b2_s[:], in_=b2[:])
    m2w0_s = cpool.tile([w_dim, ncls], F32)
    nc.sync.dma_start(out=m2w0_s[:], in_=io["m2_w0"][:])
    m2a_s = cpool.tile([128, 1], F32)
    nc.sync.dma_start(out=m2a_s[:], in_=io["m2_a"][:])
    m2w1_s = cpool.tile([ncls, ncls], F32)
    nc.sync.dma_start(out=m2w1_s[:], in_=io["m2_w1"][:])
    # prelu "abs" mode constants: c1 = (1+a)/2, c2 = (1-a)/2
    c1m1_s = cpool.tile([128, ht1], F32)
    nc.vector.tensor_scalar(c1m1_s[:], m1a_s[:], 1.0, 0.5,
                            op0=ALU.add, op1=ALU.mult)
    c2m1_s = cpool.tile([128, ht1], F32)
    nc.vector.tensor_scalar(c2m1_s[:], m1a_s[:], 1.0, -0.5,
                            op0=ALU.subtract, op1=ALU.mult)
    c1m2_s = cpool.tile([128, 1], F32)
    nc.vector.tensor_scalar(c1m2_s[:], m2a_s[:], 1.0, 0.5,
                            op0=ALU.add, op1=ALU.mult)
    c2m2_s = cpool.tile([128, 1], F32)
    nc.vector.tensor_scalar(c2m2_s[:], m2a_s[:], 1.0, -0.5,
                            op0=ALU.subtract, op1=ALU.mult)

    srcrelA_s = cpool.tile([128, CH_A], F32)
    nc.sync.dma_start(out=srcrelA_s[:], in_=io["srcrel_A"][:])
    maskbA_s = cpool.tile([128, CH_A], F32)
    nc.sync.dma_start(out=maskbA_s[:], in_=io["maskb_A"][:])
    dstrelB_s = cpool.tile([128, CH_B], F32)
    nc.sync.dma_start(out=dstrelB_s[:], in_=io["dstrel_B"][:])
    gidxB_s = cpool.tile([128, CH_B], I32)
    nc.sync.dma_start(out=gidxB_s[:], in_=io["gidx_B"][:])
    gidx16B_s = cpool.tile([128, CH_B * 8], mybir.dt.int16)
    nc.sync.dma_start(out=gidx16B_s[:], in_=io["gidx16_B"][:])
    maskbB_s = cpool.tile([128, CH_B], F32)
    nc.sync.dma_start(out=maskbB_s[:], in_=io["maskb_B"][:])

    iota_i = cpool.tile([128, 128], I32)
    nc.gpsimd.iota(iota_i[:], pattern=[[1, 128]], base=0, channel_multiplier=0)
    iota_f = cpool.tile([128, 128], F32)
    nc.vector.tensor_copy(iota_f[:], iota_i[:])
    ident = cpool.tile([128, 128], F32)
    make_identity(nc, ident[:])
    ones_s = cpool.tile([1, 128], F32)
    nc.vector.memset(ones_s[:], 1.0)
    zero_s = cpool.tile([128, max(hid, 64)], F32)
    nc.vector.memset(zero_s[:], 0.0)

    # stage 0: h1 = x @ W1 + b1
    with tc.tile_pool(name="s0", bufs=3) as sp, \
         tc.tile_pool(name="s0p", bufs=2, space="PSUM") as pp:
        for t in range(ntile):
            xt = sp.tile([128, kt1 * 128], F32, name=f"xt{t}", tag="xt")
            for i in range(kt1):
                nc.sync.dma_start(
                    out=xt[:, i * 128:(i + 1) * 128],
                    in_=xT[i * 128:(i + 1) * 128, t * 128:(t + 1) * 128])
            hp = pp.tile([128, hid], F32, name=f"hp{t}", tag="hp")
            for i in range(kt1):
                nc.tensor.matmul(hp[:], lhsT=xt[:, i * 128:(i + 1) * 128],
                                 rhs=W1_s[:, i * hid:(i + 1) * hid],
                                 start=(i == 0), stop=False)
            nc.tensor.matmul(hp[:], lhsT=ones_s[:], rhs=b1_s[:],
                             start=False, stop=True)
            hs = sp.tile([128, hid], F32, name=f"hs{t}", tag="hs")
            nc.scalar.copy(hs[:], hp[:])
            nc.sync.dma_start(out=h1_d[t * 128:(t + 1) * 128, :], in_=hs[:])

    def edge_pass(layer, passid):
        C = hid if layer == 1 else ncls
        ht = ht1 if layer == 1 else 1
        hsz = [128] * ht if layer == 1 else [ncls]
        w0_s = m1w0_s if layer == 1 else m2w0_s
        w1_s = m1w1_s if layer == 1 else m2w1_s
        a_s = m1a_s if layer == 1 else m2a_s
        c1_s = c1m1_s if layer == 1 else c1m2_s
        c2_s = c2m1_s if layer == 1 else c2m2_s
        CH = CH_A if passid == "A" else CH_B
        wmulT = io["wmulT_A"] if passid == "A" else io["wmulT_B"]
        reltab = srcrelA_s if passid == "A" else dstrelB_s
        masktab = maskbA_s if passid == "A" else maskbB_s
        seg_of_chunk = chunkA_tile if passid == "A" else chunkB_blk
        seg_off = offA if passid == "A" else offB
        seg_cnt = chA if passid == "A" else chB
        nseg = ntile if passid == "A" else nblk
        den_d = den1_d if layer == 1 else den2_d
        g_d = g1_d if layer == 1 else g2_d
        pagg_d = pagg1_d if layer == 1 else pagg2_d
        gcols = hid if layer == 1 else 64

        SLAB = 16
        ablate = meta.get("ablate", set())
        name = f"L{layer}{passid}"
        with tc.tile_pool(name=name + "w", bufs=3) as wp, \
             tc.tile_pool(name=name + "s", bufs=3) as sp, \
             tc.tile_pool(name=name + "g", bufs=3) as gp, \
             tc.tile_pool(name=name + "p1", bufs=meta.get("hidbufs", 2),
                          space="PSUM") as pp1, \
             tc.tile_pool(name=name + "p2", bufs=meta.get("owbufs", 2),
                          space="PSUM") as pp2, \
             tc.tile_pool(name=name + "pd", bufs=meta.get("segbufs", 2),
                          space="PSUM") as ppd:
            ngroups = _ceil(CH, GROUP)
            seg_psum = None
            for grp in range(ngroups):
                c0 = grp * GROUP
                gsz = min(GROUP, CH - c0)
                ne = gsz * 128
                if grp % (SLAB // GROUP) == 0:
                    slab = wp.tile([w_dim, SLAB * 128], F32,
                                   name=f"{name}slab{grp}", tag="slab")
                    ncols = min(SLAB * 128, CH * 128 - c0 * 128)
                    nc.sync.dma_start(
                        out=slab[:, :ncols],
                        in_=wmulT[:, c0 * 128: c0 * 128 + ncols])
                scol = (c0 * 128) % (SLAB * 128)
                acts = []
                for i in range(ht):
                    hp = pp1.tile([hsz[i], GROUP * 128], F32,
                                  name=f"{name}hid{grp}_{i}", tag=f"hid{i}")
                    nc.tensor.matmul(
                        hp[:, :ne],
                        lhsT=w0_s[:, i * 128: i * 128 + hsz[i]],
                        rhs=slab[:, scol: scol + ne], start=True, stop=True)
                    at = sp.tile([hsz[i], GROUP * 128], F32,
                                 name=f"{name}act{grp}_{i}", tag=f"act{i}")
                    pmode = meta.get("prelu", "dve")
                    if pmode == "abs":
                        # prelu(x,a) = c1*x + |c2*x| with c1=(1+a)/2,
                        # c2=(1-a)/2 >= 0 (host guarantees a <= 1)
                        nc.scalar.activation(at[:, :ne], hp[:, :ne], AF.Abs,
                                             scale=c2_s[: hsz[i], i: i + 1])
                        nc.vector.scalar_tensor_tensor(
                            at[:, :ne], in0=hp[:, :ne],
                            scalar=c1_s[: hsz[i], i: i + 1],
                            in1=at[:, :ne], op0=ALU.mult, op1=ALU.add)
                    elif pmode == "dve":
                        # prelu(x, a) = max(x, 0) + a * min(x, 0)
                        nc.vector.tensor_scalar(at[:, :ne], hp[:, :ne], 0.0,
                                                a_s[: hsz[i], i: i + 1],
                                                op0=ALU.min, op1=ALU.mult)
                        nc.vector.scalar_tensor_tensor(
                            at[:, :ne], in0=hp[:, :ne], scalar=0.0,
                            in1=at[:, :ne], op0=ALU.max, op1=ALU.add)
                    else:
                        fn = AF.Lrelu if pmode == "lrelu" else AF.Prelu
                        nc.scalar.activation(at[:, :ne], hp[:, :ne], fn,
                                             alpha=a_s[: hsz[i], i: i + 1])
                    acts.append(at)
                p_g = sp.tile([128, GROUP * C], F32, name=f"{name}pg{grp}",
                              tag="pg")
                for s in range(gsz):
                    ch = c0 + s
                    owp = pp2.tile([128, C], F32, name=f"{name}ow{ch}",
                                   tag="ow")
                    for i in range(ht):
                        nc.tensor.matmul(
                            owp[:],
                            lhsT=acts[i][:, s * 128:(s + 1) * 128],
                            rhs=w1_s[: hsz[i], i * C:(i + 1) * C]
                            if layer == 1 else w1_s[:],
                            start=(i == 0), stop=(i == ht - 1))
                    nc.scalar.activation(p_g[:, s * C:(s + 1) * C], owp[:],
                                         AF.Exp, bias=masktab[:, ch: ch + 1])
                oh_g = sp.tile([128, GROUP * 128], F32, name=f"{name}oh{grp}",
                               tag="ohg")
                rel3 = reltab[:, c0: c0 + gsz].to_broadcast([128, gsz, 128])
                iota3 = iota_f[:].rearrange("p (a n) -> p a n", a=1) \
                    .broadcast_to([128, gsz, 128])
                oh3 = oh_g[:, : gsz * 128].rearrange("p (g n) -> p g n",
                                                     n=128)
                nc.vector.tensor_tensor(oh3, rel3, iota3, op=ALU.is_equal)
                if passid == "B" and "nogather" not in ablate:
                    gt = gp.tile([128, GROUP * gcols], F32,
                                 name=f"{name}gt{grp}", tag="gt")
                    if "oldgather" in ablate:
                        nc.gpsimd.indirect_dma_start(
                            out=gt[:, : gsz * gcols], out_offset=None,
                            in_=g_d[:],
                            in_offset=bass.IndirectOffsetOnAxis(
                                ap=gidxB_s[:, c0: c0 + gsz], axis=0))
                    else:
                        nc.gpsimd.dma_gather(
                            gt[:, : gsz * gcols].rearrange(
                                "p (g c) -> p g c", c=gcols),
                            g_d[:],
                            gidx16B_s[:, c0 * 8: c0 * 8 + gsz * 8],
                            num_idxs=gsz * 128,
                            num_idxs_reg=gsz * 128,
                            elem_size=gcols)
                    msg_g = sp.tile([128, GROUP * C], F32,
                                    name=f"{name}msg{grp}", tag="msg")
                    m3 = msg_g[:, : gsz * C].rearrange("p (g c) -> p g c",
                                                       c=C)
                    pg3 = p_g[:, : gsz * C].rearrange("p (g c) -> p g c", c=C)
                    gt3 = gt[:, : gsz * gcols].rearrange(
                        "p (g c) -> p g c", c=gcols)[:, :, :C]
                    nc.vector.tensor_tensor(m3, pg3, gt3, op=ALU.mult)
                    rhs_g = msg_g
                else:
                    rhs_g = p_g
                for s in range(gsz):
                    ch = c0 + s
                    seg = seg_of_chunk[ch]
                    first = (ch == seg_off[seg])
                    last = (ch == seg_off[seg] + seg_cnt[seg] - 1)
                    if first:
                        seg_psum = ppd.tile([128, C], F32,
                                            name=f"{name}seg{seg}", tag="seg")
                    if "noseg" in ablate:
                        continue
                    nc.tensor.matmul(seg_psum[:],
                                     lhsT=oh_g[:, s * 128:(s + 1) * 128],
                                     rhs=rhs_g[:, s * C:(s + 1) * C],
                                     start=first, stop=last)
                    if last:
                        st = sp.tile([128, C], F32, name=f"{name}st{seg}",
                                     tag="st")
                        if meta.get("stcopy", "dve") == "act":
                            nc.scalar.copy(st[:], seg_psum[:])
                        else:
                            nc.vector.tensor_copy(st[:], seg_psum[:])
                        tgt = den_d if passid == "A" else pagg_d
                        nc.sync.dma_start(
                            out=tgt[seg * 128:(seg + 1) * 128, :], in_=st[:])
            if passid == "B":
                for b in range(nseg):
                    if seg_cnt[b] == 0:
                        nc.sync.dma_start(
                            out=pagg_d[b * 128:(b + 1) * 128, :],
                            in_=zero_s[:, :C])

    def g_stage(layer):
        C = hid if layer == 1 else ncls
        gcols = hid if layer == 1 else 64
        h_d = h1_d if layer == 1 else h2_d
        den_d = den1_d if layer == 1 else den2_d
        g_d = g1_d if layer == 1 else g2_d
        with tc.tile_pool(name=f"gs{layer}", bufs=3) as sp:
            for t in range(ntile):
                hsb = sp.tile([128, C], F32, name=f"gh{t}", tag="gh")
                nc.sync.dma_start(out=hsb[:],
                                  in_=h_d[t * 128:(t + 1) * 128, :C])
                dsb = sp.tile([128, C], F32, name=f"gd{t}", tag="gd")
                nc.sync.dma_start(out=dsb[:],
                                  in_=den_d[t * 128:(t + 1) * 128, :C])
                nc.vector.tensor_scalar_max(dsb[:], dsb[:], 1e-30)
                rsb = sp.tile([128, gcols], F32, name=f"gr{t}", tag="gr")
                if gcols > C:
                    nc.vector.memset(rsb[:], 0.0)
                nc.vector.reciprocal(rsb[:, :C], dsb[:])
                nc.vector.tensor_tensor(rsb[:, :C], rsb[:, :C], hsb[:],
                                        op=ALU.mult)
                nc.sync.dma_start(out=g_d[t * 128:(t + 1) * 128, :],
                                  in_=rsb[:])

    ablate = meta.get("ablate", set())
    if "noA" not in ablate:
        edge_pass(1, "A")
    g_stage(1)
    if "noB" not in ablate:
        edge_pass(1, "B")
    # L2 pass A only needs w_mul + L2 MLP weights -> emit it before the
    # ReduceScatter so compute hides the collective.
    if "noA" not in ablate:
        edge_pass(2, "A")
    if "noRS" not in ablate:
        nc.gpsimd.collective_compute(
            "ReduceScatter", ALU.add, replica_groups=rg,
            ins=[pagg1_d.opt()], outs=[agg1_d.opt()])

    # h2 = elu(agg1) @ W2 + b2
    with tc.tile_pool(name="h2s", bufs=3) as sp, \
         tc.tile_pool(name="h2p", bufs=2, space="PSUM") as pp, \
         tc.tile_pool(name="h2pt", bufs=2, space="PSUM") as pt:
        kt2 = hid // 128
        for t in range(ntile):
            ea = sp.tile([128, hid], F32, name=f"ea{t}", tag="ea")
            nc.sync.dma_start(out=ea[:], in_=agg1_d[t * 128:(t + 1) * 128, :])
            t0 = sp.tile([128, hid], F32, name=f"t0{t}", tag="t0")
            nc.vector.tensor_scalar_min(t0[:], ea[:], 0.0)
            nc.scalar.activation(t0[:], t0[:], AF.Exp)
            nc.vector.tensor_scalar_add(t0[:], t0[:], -1.0)
            nc.vector.scalar_tensor_tensor(ea[:], in0=ea[:], scalar=0.0,
                                           in1=t0[:], op0=ALU.max,
                                           op1=ALU.add)
            h2p = pp.tile([128, ncls], F32, name=f"h2p{t}", tag="h2p")
            for i in range(kt2):
                tp = pt.tile([128, 128], F32, name=f"tp{t}_{i}", tag="tp")
                nc.tensor.transpose(tp[:], ea[:, i * 128:(i + 1) * 128],
                                    ident[:])
                ts = sp.tile([128, 128], F32, name=f"ts{t}_{i}", tag="ts")
                nc.vector.tensor_copy(ts[:], tp[:])
                nc.tensor.matmul(h2p[:], lhsT=ts[:],
                                 rhs=W2_s[:, i * ncls:(i + 1) * ncls],
                                 start=(i == 0), stop=False)
            nc.tensor.matmul(h2p[:], lhsT=ones_s[:], rhs=b2_s[:],
                             start=False, stop=True)
            h2s = sp.tile([128, 64], F32, name=f"h2s{t}", tag="h2s")
            nc.vector.memset(h2s[:], 0.0)
            nc.vector.tensor_copy(h2s[:, :ncls], h2p[:])
            nc.sync.dma_start(out=h2_d[t * 128:(t + 1) * 128, :], in_=h2s[:])

    g_stage(2)
    if "noB" not in ablate:
        edge_pass(2, "B")
    if "noRS" not in ablate:
        nc.gpsimd.collective_compute(
            "ReduceScatter", ALU.add, replica_groups=rg,
            ins=[pagg2_d.opt()], outs=[agg2_d.opt()])

    # log_softmax
    with tc.tile_pool(name="ls", bufs=3) as sp:
        for t in range(ntile):
            xt = sp.tile([128, ncls], F32, name=f"lx{t}", tag="lx")
            nc.sync.dma_start(out=xt[:], in_=agg2_d[t * 128:(t + 1) * 128, :])
            mx = sp.tile([128, 1], F32, name=f"lm{t}", tag="lm")
            nc.vector.reduce_max(mx[:], xt[:], axis=mybir.AxisListType.X)
            nmx = sp.tile([128, 1], F32, name=f"lnm{t}", tag="lnm")
            nc.vector.tensor_scalar_mul(nmx[:], mx[:], -1.0)
            ex = sp.tile([128, ncls], F32, name=f"le{t}", tag="le")
            sm = sp.tile([128, 1], F32, name=f"lsm{t}", tag="lsm")
            nc.scalar.activation(ex[:], xt[:], AF.Exp, bias=nmx[:, :1],
                                 accum_out=sm[:])
            lg = sp.tile([128, 1], F32, name=f"lg{t}", tag="lg")
            nc.scalar.activation(lg[:], sm[:], AF.Ln)
            ot = sp.tile([128, ncls], F32, name=f"lo{t}", tag="lo")
            nc.vector.tensor_scalar(ot[:], xt[:], mx[:, :1], lg[:, :1],
                                    op0=ALU.subtract, op1=ALU.subtract)
            nc.sync.dma_start(out=out[t * 128:(t + 1) * 128, :], in_=ot[:])


# ----------------------------------------------------------------------------
# entry point
# ----------------------------------------------------------------------------

def kernel(**inputs) -> np.ndarray:
    from concourse import bass_utils
    meta, in_maps = build_meta_and_inputs(inputs, NCORES)
    nc = build_kernel(meta)
    res = bass_utils.run_bass_kernel_spmd(
        nc, in_maps, core_ids=list(range(NCORES)))
    got = np.concatenate([res.results[k]["out"] for k in range(NCORES)])
    return np.ascontiguousarray(got[: meta["n_nodes"]]).astype(np.float32)
